# revision 9
# baseline (speedup 1.0000x reference)
"""Causal self-attention (B=4, T=2048, HID=2048, NH=16, HD=128) on 8 TRN2 cores.

Tensor-parallel over heads (2 heads/core). v2 redesign vs baseline:
  - No denominator matmuls on PE: P blocks land in a per-j slab
    [128, nk, 512]; den = DVE strided reduce over k-blocks + GPSIMD
    partition_all_reduce (broadcast f32), reciprocal_approx_fast, one mul.
  - K's RMSNorm folded into the Exp activation's per-partition scale
    (rstd_k AP), so kT is stored un-normalized; q gets rstd_q*wqk*sqrt(HD).
  - rstd = exp(-0.5*ln(ssq)) on ScalarE: Ln/Exp/Copy all live in one
    activation table -> no ACT_TABLE_LOAD thrash (was 145us).
  - Per-group (4 t-block) batched DVE rope/square/reduce in fp16.
  - Fused transpose evacuation: 4 PE transposes into one [128,4,128] psum
    tile, single strided copy into the combined qkT resident.
  - proj matmuls of batch b-1 interleaved into attn(b)'s exp-paced PE
    bubbles; aT is double-buffered by batch parity to keep this safe.
  - y emitted fp16 (host sums partials in f32).
"""

import sys

if "/opt/trn_rl_repo" not in sys.path:
    sys.path.insert(0, "/opt/trn_rl_repo")

from contextlib import ExitStack

import numpy as np

import concourse.bass as bass
import concourse.tile as tile
from concourse import bacc, mybir
from concourse.bass_utils import run_bass_kernel_spmd

F32 = mybir.dt.float32
F16 = mybir.dt.float16
AF = mybir.ActivationFunctionType
ALU = mybir.AluOpType
AX = mybir.AxisListType
RED = bass.bass_isa.ReduceOp

B, T, HID = 4, 2048, 2048
NH, HD = 16, 128
N_CORES = 8
NHC = NH // N_CORES          # heads per core = 2
NC = NHC * HD                # per-core head cols = 256
TM = B * T
TBB = T // 128               # 16 t-blocks per batch
KC = HID // 128              # 16 contraction chunks
ROPE_BASE = 10000.0
EXP_BIAS = -1.25


def build_program():
    nc = bacc.Bacc("TRN2", target_bir_lowering=False, debug=False,
                   num_devices=N_CORES)

    xT = nc.dram_tensor("xT", [HID, TM], F16, kind="ExternalInput").ap()
    wqkvd = nc.dram_tensor("wqkv", [HID, 2 * NC + NC], F16,
                           kind="ExternalInput").ap()
    wod = nc.dram_tensor("wo", [NC, HID], F16, kind="ExternalInput").ap()
    cosd = nc.dram_tensor("cos", [T, HD // 2], F16, kind="ExternalInput").ap()
    sind = nc.dram_tensor("sin", [T, HD // 2], F16, kind="ExternalInput").ap()
    w2d = nc.dram_tensor("w2", [128, NC], F16, kind="ExternalInput").ap()
    maskd = nc.dram_tensor("masks", [4, 128, 512], F16, kind="ExternalInput").ap()
    identd = nc.dram_tensor("ident", [128, 128], F16, kind="ExternalInput").ap()
    y = nc.dram_tensor("y", [HID, TM], F16, kind="ExternalOutput").ap()

    with tile.TileContext(nc) as tc, ExitStack() as ctx:
        consts = ctx.enter_context(tc.tile_pool(name="consts", bufs=1))

        wqkv_sb = consts.tile([128, KC, 3 * NC], F16, tag="wqkv")
        nc.sync.dma_start(
            out=wqkv_sb, in_=wqkvd.rearrange("(k1 k2) n -> k2 k1 n", k2=128))
        ident = consts.tile([128, 128], F16, tag="ident")
        nc.sync.dma_start(out=ident, in_=identd)
        cos_sb = consts.tile([128, TBB, HD // 2], F16, tag="cos")
        sin_sb = consts.tile([128, TBB, HD // 2], F16, tag="sin")
        nc.sync.dma_start(out=cos_sb,
                          in_=cosd.rearrange("(t1 t2) j -> t2 t1 j", t2=128))
        nc.sync.dma_start(out=sin_sb,
                          in_=sind.rearrange("(t1 t2) j -> t2 t1 j", t2=128))
        w2_sb = consts.tile([128, NC], F16, tag="w2")
        nc.sync.dma_start(out=w2_sb, in_=w2d)
        mask_sb = consts.tile([128, 4, 512], F16, tag="mask")
        nc.sync.dma_start(out=mask_sb, in_=maskd.rearrange("m p t -> p m t"))
        wo_sb = consts.tile([128, NHC, HID], F16, tag="wo")
        nc.sync.dma_start(
            out=wo_sb, in_=wod.rearrange("(n1 n2) c -> n2 n1 c", n2=128))
        zero_b = consts.tile([128, 1], F32, tag="zb")
        nc.vector.memset(zero_b, 0.0)
        negc = consts.tile([128, 1], F32, tag="negc")
        nc.vector.memset(negc, EXP_BIAS)

        ps_qkv = ctx.enter_context(tc.tile_pool(name="ps_qkv", bufs=2, space="PSUM"))
        ps_tr = ctx.enter_context(tc.tile_pool(name="ps_tr", bufs=1, space="PSUM"))
        ps_st = ctx.enter_context(tc.tile_pool(name="ps_st", bufs=2, space="PSUM"))
        ps_acc = ctx.enter_context(tc.tile_pool(name="ps_acc", bufs=2, space="PSUM"))
        ps_y = ctx.enter_context(tc.tile_pool(name="ps_y", bufs=1, space="PSUM"))

        res = ctx.enter_context(tc.tile_pool(name="res", bufs=1))
        xt_pool = ctx.enter_context(tc.tile_pool(name="xt", bufs=2))
        g_pool = ctx.enter_context(tc.tile_pool(name="gp", bufs=2))
        slab_pool = ctx.enter_context(tc.tile_pool(name="slab", bufs=2))
        den_pool = ctx.enter_context(tc.tile_pool(name="den", bufs=2))
        y_pool = ctx.enter_context(tc.tile_pool(name="yo", bufs=3))

        proj_jobs = []

        def emit_proj_job(job):
            bb, cb, tg, aT = job
            y_ps = ps_y.tile([128, 512], F32, tag="yacc",
                             name=f"yps{bb}_{cb}_{tg}")
            for n in range(NHC):
                nc.tensor.matmul(y_ps, wo_sb[:, n, bass.ts(cb, 128)],
                                 aT[:, n, bass.ds(tg * 512, 512)],
                                 start=(n == 0), stop=(n == NHC - 1))
            ysb = y_pool.tile([128, 512], F16, tag="ysb",
                              name=f"ysb{bb}_{cb}_{tg}")
            nc.any.tensor_copy(ysb, y_ps)
            nc.sync.dma_start(
                out=y[bass.ts(cb, 128), bass.ds(bb * T + tg * 512, 512)],
                in_=ysb)

        def emit_group_transposes(pend, qkT):
            rot, nrmq, g = pend
            for sub in range(4):
                tbl = 4 * g + sub
                t_ps = ps_tr.tile([128, 4, 128], F16, tag="tr",
                                  name=f"tps{tbl}")
                nc.tensor.transpose(t_ps[:, 0, :], nrmq[:, sub, 0, :], ident)
                nc.tensor.transpose(t_ps[:, 1, :], nrmq[:, sub, 1, :], ident)
                nc.tensor.transpose(
                    t_ps[:, 2, :],
                    rot[:, sub, 2, :, :].rearrange("p h d -> p (h d)"), ident)
                nc.tensor.transpose(
                    t_ps[:, 3, :],
                    rot[:, sub, 3, :, :].rearrange("p h d -> p (h d)"), ident)
                nc.any.tensor_copy(qkT[:, :, bass.ds(tbl * 128, 128)], t_ps)

        def qkv_group(b, g, qkT, v_t, rstdk, pending):
            xt = xt_pool.tile([128, KC, 512], F16, tag="xt")
            nc.sync.dma_start(
                out=xt,
                in_=xT[:, bass.ds((b * TBB + 4 * g) * 128, 512)]
                .rearrange("(k1 k2) t -> k2 k1 t", k2=128))
            qk16 = g_pool.tile([128, 4, 512], F16, tag="qk16")
            for sub in range(4):
                tbl = 4 * g + sub
                qk_ps = ps_qkv.tile([128, 512], F32, tag="ps")
                v_ps = ps_qkv.tile([128, NC], F32, tag="ps")
                for k1 in range(KC):
                    lhs = xt[:, k1, bass.ts(sub, 128)]
                    st, sp = (k1 == 0), (k1 == KC - 1)
                    nc.tensor.matmul(qk_ps, lhs,
                                     wqkv_sb[:, k1, 0:512], start=st, stop=sp)
                    nc.tensor.matmul(v_ps, lhs,
                                     wqkv_sb[:, k1, 512:768], start=st, stop=sp)
                nc.scalar.copy(qk16[:, sub, :], qk_ps)
                nc.scalar.copy(v_t[:, tbl, :], v_ps)
            if pending[0] is not None:
                emit_group_transposes(pending[0], qkT)
                pending[0] = None

            # rope on all 4 t-blocks at once, fp16
            rot = g_pool.tile([128, 4, 4, 2, HD // 2], F16, tag="rot")
            tmp = g_pool.tile([128, 4, 4, HD // 2], F16, tag="tmp")
            v5 = qk16.rearrange("p t (g h d) -> p t g h d", g=4, h=2)
            x1, x2 = v5[:, :, :, 0, :], v5[:, :, :, 1, :]
            ct = cos_sb[:, 4 * g:4 * g + 4, None, :].broadcast_to(
                [128, 4, 4, HD // 2])
            sn = sin_sb[:, 4 * g:4 * g + 4, None, :].broadcast_to(
                [128, 4, 4, HD // 2])
            r1 = rot[:, :, :, 0, :]
            r2 = rot[:, :, :, 1, :]
            nc.vector.tensor_mul(r1, x1, ct)
            nc.vector.tensor_mul(tmp, x2, sn)
            nc.vector.tensor_sub(r1, r1, tmp)
            nc.vector.tensor_mul(r2, x2, ct)
            nc.vector.tensor_mul(tmp, x1, sn)
            nc.vector.tensor_add(r2, r2, tmp)

            # ssq per (t-block, tensor-group); rstd = exp(-0.5*ln(ssq))
            sq = g_pool.tile([128, 4, 4, HD], F16, tag="sq")
            rfull = rot.rearrange("p t g h d -> p t g (h d)")
            nc.vector.tensor_mul(sq, rfull, rfull)
            ssq = g_pool.tile([128, 16], F32, tag="ssq")
            nc.vector.tensor_reduce(
                ssq, sq.rearrange("p t g d -> p (t g) d"),
                axis=AX.X, op=ALU.add)
            lssq = g_pool.tile([128, 16], F32, tag="lssq")
            nc.scalar.activation(lssq, ssq, AF.Ln, bias=zero_b)
            lv = lssq.rearrange("p (t g) -> p t g", g=4)
            rstdq = g_pool.tile([128, 4, 2], F32, tag="rstdq")
            nc.scalar.activation(rstdq, lv[:, :, 0:2], AF.Exp, scale=-0.5,
                                 bias=zero_b)
            nc.scalar.activation(rstdk[:, 4 * g:4 * g + 4, :], lv[:, :, 2:4],
                                 AF.Exp, scale=-0.5, bias=zero_b)

            # q_hat = rope(q) * rstd_q * (wq*wk*sqrt(HD))
            nrmq = g_pool.tile([128, 4, 2, HD], F16, tag="nrmq")
            rq = rot[:, :, 0:2, :, :].rearrange("p t g h d -> p t g (h d)")
            nc.vector.tensor_mul(
                nrmq, rq, rstdq[:, :, :, None].broadcast_to([128, 4, 2, HD]))
            nc.vector.tensor_mul(
                nrmq, nrmq,
                w2_sb.rearrange("p (g d) -> p g d", g=2)[:, None, :, :]
                .broadcast_to([128, 4, 2, HD]))
            pending[0] = (rot, nrmq, g)

        def attn(b, h, qkT, v_t, rstdk, aT):
            for j in range(T // 512):
                nk = 4 * j + 4
                slab = slab_pool.tile([128, TBB, 512], F16, tag="slab",
                                      name=f"slab{b}_{h}_{j}")
                outT = ps_acc.tile([128, 512], F32, tag="acc",
                                   name=f"outT{b}_{h}_{j}")
                qrhs = qkT[:, h, bass.ds(j * 512, 512)]
                for k in range(nk):
                    st_ps = ps_st.tile([128, 512], F32, tag="st")
                    nc.tensor.matmul(st_ps, qkT[:, 2 + h, bass.ts(k, 128)],
                                     qrhs, start=True, stop=True)
                    nc.scalar.activation(slab[:, k, :], st_ps, AF.Exp,
                                         bias=negc,
                                         scale=rstdk[:, k, h:h + 1])
                    if k >= 4 * j:
                        nc.vector.tensor_mul(slab[:, k, :], slab[:, k, :],
                                             mask_sb[:, k - 4 * j, :])
                    if k >= 1:
                        nc.tensor.matmul(outT, v_t[:, k - 1, bass.ds(h * HD, HD)],
                                         slab[:, k - 1, :],
                                         start=(k == 1), stop=False)
                    if proj_jobs:
                        emit_proj_job(proj_jobs.pop(0))
                nc.tensor.matmul(outT, v_t[:, nk - 1, bass.ds(h * HD, HD)],
                                 slab[:, nk - 1, :], start=False, stop=True)
                # denominator: sum over k-blocks (DVE) then partitions (GPSIMD)
                ksum = den_pool.tile([128, 512], F32, tag="ks")
                nc.vector.tensor_reduce(
                    ksum, slab[:, 0:nk, :].rearrange("p k t -> p t k"),
                    axis=AX.X, op=ALU.add)
                den = den_pool.tile([128, 512], F32, tag="bc")
                nc.gpsimd.partition_all_reduce(den, ksum, channels=128,
                                               reduce_op=RED.add)
                rec = den_pool.tile([128, 512], F32, tag="rec")
                nc.vector.reciprocal_approx_fast(rec, den)
                nc.vector.tensor_mul(aT[:, h, bass.ds(j * 512, 512)], outT, rec)

        for b in range(B):
            qkT = res.tile([128, 4, T], F16, name=f"qkT{b}", tag="qkT")
            v_t = res.tile([128, TBB, NC], F16, name=f"v{b}", tag="v")
            rstdk = res.tile([128, TBB, NHC], F32, name=f"rstdk{b}", tag="rstdk")
            aT = res.tile([128, NHC, T], F16, name=f"aT{b}", tag=f"aT{b % 2}")
            pending = [None]
            for g in range(TBB // 4):
                qkv_group(b, g, qkT, v_t, rstdk, pending)
            if pending[0] is not None:
                emit_group_transposes(pending[0], qkT)
                pending[0] = None
            for h in range(NHC):
                attn(b, h, qkT, v_t, rstdk, aT)
            for cb in range(HID // 128):
                for tg in range(4):
                    proj_jobs.append((b, cb, tg, aT))
        while proj_jobs:
            emit_proj_job(proj_jobs.pop(0))

    nc.compile()
    return nc


_CACHE = {}


def _get_program():
    if "nc" not in _CACHE:
        _CACHE["nc"] = build_program()
    return _CACHE["nc"]


def _host_tables():
    inv = 1.0 / (ROPE_BASE ** (np.arange(0, HD, 2, dtype=np.float32) / HD))
    freqs = np.arange(T, dtype=np.float32)[:, None] * inv[None, :]
    cos = np.cos(freqs).astype(np.float16)
    sin = np.sin(freqs).astype(np.float16)
    m = np.zeros((4, 128, 512), dtype=np.float16)
    s_idx = np.arange(128)[:, None]
    t_idx = np.arange(512)[None, :]
    for off in range(4):
        m[off] = ((off * 128 + s_idx) <= t_idx).astype(np.float16)
    return cos, sin, m


def kernel(x, Wq, Wk, Wv, Wo, q_rms_w, k_rms_w, **_):
    nc = _get_program()
    cos, sin, masks = _host_tables()
    xT = np.ascontiguousarray(
        np.asarray(x, dtype=np.float32).reshape(TM, HID).T).astype(np.float16)
    w2 = (np.asarray(q_rms_w, dtype=np.float32)
          * np.asarray(k_rms_w, dtype=np.float32) * np.sqrt(HD))
    w2_b = np.ascontiguousarray(
        np.broadcast_to(np.tile(w2, NHC)[None, :], (128, NC))).astype(np.float16)
    ident_h = np.eye(128, dtype=np.float16)

    in_maps = []
    for c in range(N_CORES):
        cols = slice(c * NC, (c + 1) * NC)
        in_maps.append({
            "xT": xT,
            "wqkv": np.ascontiguousarray(
                np.concatenate([Wq[:, cols], Wk[:, cols], Wv[:, cols]], axis=1)
            ).astype(np.float16),
            "wo": np.ascontiguousarray(Wo[cols, :]).astype(np.float16),
            "cos": cos, "sin": sin, "w2": w2_b, "masks": masks,
            "ident": ident_h,
        })

    res = run_bass_kernel_spmd(nc, in_maps, list(range(N_CORES)))
    out = res.results[0]["y"].astype(np.float32)
    for c in range(1, N_CORES):
        out += res.results[c]["y"]
    return np.ascontiguousarray(out.T).reshape(B, T, HID).astype(np.float32)


# revision 11
# speedup vs baseline: 1.2124x; 1.2124x over previous
"""Causal self-attention (B=4, T=2048, HID=2048, NH=16, HD=128) on 8 TRN2 cores.

Tensor-parallel over heads (2 heads/core). v2 redesign vs baseline:
  - No denominator matmuls on PE: P blocks land in a per-j slab
    [128, nk, 512]; den = DVE strided reduce over k-blocks + GPSIMD
    partition_all_reduce (broadcast f32), reciprocal_approx_fast, one mul.
  - K's RMSNorm folded into the Exp activation's per-partition scale
    (rstd_k AP), so kT is stored un-normalized; q gets rstd_q*wqk*sqrt(HD).
  - rstd = exp(-0.5*ln(ssq)) on ScalarE: Ln/Exp/Copy all live in one
    activation table -> no ACT_TABLE_LOAD thrash (was 145us).
  - Per-group (4 t-block) batched DVE rope/square/reduce in fp16.
  - Fused transpose evacuation: 4 PE transposes into one [128,4,128] psum
    tile, single strided copy into the combined qkT resident.
  - proj matmuls of batch b-1 interleaved into attn(b)'s exp-paced PE
    bubbles; aT is double-buffered by batch parity to keep this safe.
  - y emitted fp16 (host sums partials in f32).
"""

import sys

if "/opt/trn_rl_repo" not in sys.path:
    sys.path.insert(0, "/opt/trn_rl_repo")

from contextlib import ExitStack

import numpy as np

import concourse.bass as bass
import concourse.tile as tile
from concourse import bacc, mybir
from concourse.bass_utils import run_bass_kernel_spmd

F32 = mybir.dt.float32
F16 = mybir.dt.float16
AF = mybir.ActivationFunctionType
ALU = mybir.AluOpType
AX = mybir.AxisListType
RED = bass.bass_isa.ReduceOp

B, T, HID = 4, 2048, 2048
NH, HD = 16, 128
N_CORES = 8
NHC = NH // N_CORES          # heads per core = 2
NC = NHC * HD                # per-core head cols = 256
TM = B * T
TBB = T // 128               # 16 t-blocks per batch
KC = HID // 128              # 16 contraction chunks
ROPE_BASE = 10000.0
EXP_BIAS = -1.25


def build_program():
    nc = bacc.Bacc("TRN2", target_bir_lowering=False, debug=False,
                   num_devices=N_CORES)

    xT = nc.dram_tensor("xT", [HID, TM], F16, kind="ExternalInput").ap()
    wqkvd = nc.dram_tensor("wqkv", [HID, 2 * NC + NC], F16,
                           kind="ExternalInput").ap()
    wod = nc.dram_tensor("wo", [NC, HID], F16, kind="ExternalInput").ap()
    cosd = nc.dram_tensor("cos", [T, HD // 2], F16, kind="ExternalInput").ap()
    sind = nc.dram_tensor("sin", [T, HD // 2], F16, kind="ExternalInput").ap()
    w2d = nc.dram_tensor("w2", [128, NC], F16, kind="ExternalInput").ap()
    maskd = nc.dram_tensor("masks", [4, 128, 512], F16, kind="ExternalInput").ap()
    identd = nc.dram_tensor("ident", [128, 128], F16, kind="ExternalInput").ap()
    y = nc.dram_tensor("y", [HID, TM], F16, kind="ExternalOutput").ap()

    with tile.TileContext(nc) as tc, ExitStack() as ctx:
        consts = ctx.enter_context(tc.tile_pool(name="consts", bufs=1))

        wqkv_sb = consts.tile([128, KC, 3 * NC], F16, tag="wqkv")
        nc.sync.dma_start(
            out=wqkv_sb, in_=wqkvd.rearrange("(k1 k2) n -> k2 k1 n", k2=128))
        ident = consts.tile([128, 128], F16, tag="ident")
        nc.sync.dma_start(out=ident, in_=identd)
        cos_sb = consts.tile([128, TBB, HD // 2], F16, tag="cos")
        sin_sb = consts.tile([128, TBB, HD // 2], F16, tag="sin")
        nc.sync.dma_start(out=cos_sb,
                          in_=cosd.rearrange("(t1 t2) j -> t2 t1 j", t2=128))
        nc.sync.dma_start(out=sin_sb,
                          in_=sind.rearrange("(t1 t2) j -> t2 t1 j", t2=128))
        w2_sb = consts.tile([128, NC], F16, tag="w2")
        nc.sync.dma_start(out=w2_sb, in_=w2d)
        mask_sb = consts.tile([128, 4, 512], F16, tag="mask")
        nc.sync.dma_start(out=mask_sb, in_=maskd.rearrange("m p t -> p m t"))
        wo_sb = consts.tile([128, NHC, HID], F16, tag="wo")
        nc.sync.dma_start(
            out=wo_sb, in_=wod.rearrange("(n1 n2) c -> n2 n1 c", n2=128))
        zero_b = consts.tile([128, 1], F32, tag="zb")
        nc.vector.memset(zero_b, 0.0)
        negc = consts.tile([128, 1], F32, tag="negc")
        nc.vector.memset(negc, EXP_BIAS)
        ones_bc = consts.tile([128, 128], F16, tag="ones")
        nc.vector.memset(ones_bc, 1.0)

        ps_qkv = ctx.enter_context(tc.tile_pool(name="ps_qkv", bufs=2, space="PSUM"))
        ps_tr = ctx.enter_context(tc.tile_pool(name="ps_tr", bufs=1, space="PSUM"))
        ps_st = ctx.enter_context(tc.tile_pool(name="ps_st", bufs=2, space="PSUM"))
        ps_acc = ctx.enter_context(tc.tile_pool(name="ps_acc", bufs=2, space="PSUM"))
        ps_y = ctx.enter_context(tc.tile_pool(name="ps_y", bufs=1, space="PSUM"))

        res = ctx.enter_context(tc.tile_pool(name="res", bufs=1))
        xt_pool = ctx.enter_context(tc.tile_pool(name="xt", bufs=2))
        g_pool = ctx.enter_context(tc.tile_pool(name="gp", bufs=2))
        slab_pool = ctx.enter_context(tc.tile_pool(name="slab", bufs=2))
        den_pool = ctx.enter_context(tc.tile_pool(name="den", bufs=2))
        y_pool = ctx.enter_context(tc.tile_pool(name="yo", bufs=3))

        proj_jobs = []

        def emit_proj_job(job):
            bb, cb, tg, aT = job
            y_ps = ps_y.tile([128, 512], F32, tag="yacc",
                             name=f"yps{bb}_{cb}_{tg}")
            for n in range(NHC):
                nc.tensor.matmul(y_ps, wo_sb[:, n, bass.ts(cb, 128)],
                                 aT[:, n, bass.ds(tg * 512, 512)],
                                 start=(n == 0), stop=(n == NHC - 1))
            ysb = y_pool.tile([128, 512], F16, tag="ysb",
                              name=f"ysb{bb}_{cb}_{tg}")
            nc.any.tensor_copy(ysb, y_ps)
            nc.sync.dma_start(
                out=y[bass.ts(cb, 128), bass.ds(bb * T + tg * 512, 512)],
                in_=ysb)

        def emit_group_transposes(pend, qkT):
            rot, nrmq, g = pend
            for sub in range(4):
                tbl = 4 * g + sub
                t_ps = ps_tr.tile([128, 4, 128], F16, tag="tr",
                                  name=f"tps{tbl}")
                nc.tensor.transpose(t_ps[:, 0, :], nrmq[:, sub, 0, :], ident)
                nc.tensor.transpose(t_ps[:, 1, :], nrmq[:, sub, 1, :], ident)
                nc.tensor.transpose(
                    t_ps[:, 2, :],
                    rot[:, sub, 2, :, :].rearrange("p h d -> p (h d)"), ident)
                nc.tensor.transpose(
                    t_ps[:, 3, :],
                    rot[:, sub, 3, :, :].rearrange("p h d -> p (h d)"), ident)
                nc.any.tensor_copy(qkT[:, :, bass.ds(tbl * 128, 128)], t_ps)

        def qkv_group(b, g, qkT, v_t, rstdk, pending):
            xt = xt_pool.tile([128, KC, 512], F16, tag="xt")
            nc.sync.dma_start(
                out=xt,
                in_=xT[:, bass.ds((b * TBB + 4 * g) * 128, 512)]
                .rearrange("(k1 k2) t -> k2 k1 t", k2=128))
            qk16 = g_pool.tile([128, 4, 512], F16, tag="qk16")
            for sub in range(4):
                tbl = 4 * g + sub
                qk_ps = ps_qkv.tile([128, 512], F32, tag="ps")
                v_ps = ps_qkv.tile([128, NC], F32, tag="ps")
                for k1 in range(KC):
                    lhs = xt[:, k1, bass.ts(sub, 128)]
                    st, sp = (k1 == 0), (k1 == KC - 1)
                    nc.tensor.matmul(qk_ps, lhs,
                                     wqkv_sb[:, k1, 0:512], start=st, stop=sp)
                    nc.tensor.matmul(v_ps, lhs,
                                     wqkv_sb[:, k1, 512:768], start=st, stop=sp)
                nc.scalar.copy(qk16[:, sub, :], qk_ps)
                nc.scalar.copy(v_t[:, tbl, :], v_ps)
            if pending[0] is not None:
                emit_group_transposes(pending[0], qkT)
                pending[0] = None

            # rope on all 4 t-blocks at once, fp16
            rot = g_pool.tile([128, 4, 4, 2, HD // 2], F16, tag="rot")
            tmp = g_pool.tile([128, 4, 4, HD // 2], F16, tag="tmp")
            v5 = qk16.rearrange("p t (g h d) -> p t g h d", g=4, h=2)
            x1, x2 = v5[:, :, :, 0, :], v5[:, :, :, 1, :]
            ct = cos_sb[:, 4 * g:4 * g + 4, None, :].broadcast_to(
                [128, 4, 4, HD // 2])
            sn = sin_sb[:, 4 * g:4 * g + 4, None, :].broadcast_to(
                [128, 4, 4, HD // 2])
            r1 = rot[:, :, :, 0, :]
            r2 = rot[:, :, :, 1, :]
            nc.vector.tensor_mul(r1, x1, ct)
            nc.vector.tensor_mul(tmp, x2, sn)
            nc.vector.tensor_sub(r1, r1, tmp)
            nc.vector.tensor_mul(r2, x2, ct)
            nc.vector.tensor_mul(tmp, x1, sn)
            nc.vector.tensor_add(r2, r2, tmp)

            # ssq per (t-block, tensor-group); rstd = exp(-0.5*ln(ssq))
            sq = g_pool.tile([128, 4, 4, HD], F16, tag="sq")
            rfull = rot.rearrange("p t g h d -> p t g (h d)")
            nc.vector.tensor_mul(sq, rfull, rfull)
            ssq = g_pool.tile([128, 16], F32, tag="ssq")
            nc.vector.tensor_reduce(
                ssq, sq.rearrange("p t g d -> p (t g) d"),
                axis=AX.X, op=ALU.add)
            lssq = g_pool.tile([128, 16], F32, tag="lssq")
            nc.scalar.activation(lssq, ssq, AF.Ln, bias=zero_b)
            lv = lssq.rearrange("p (t g) -> p t g", g=4)
            rstdq = g_pool.tile([128, 4, 2], F32, tag="rstdq")
            nc.scalar.activation(rstdq, lv[:, :, 0:2], AF.Exp, scale=-0.5,
                                 bias=zero_b)
            nc.scalar.activation(rstdk[:, 4 * g:4 * g + 4, :], lv[:, :, 2:4],
                                 AF.Exp, scale=-0.5, bias=zero_b)

            # q_hat = rope(q) * rstd_q * (wq*wk*sqrt(HD))
            nrmq = g_pool.tile([128, 4, 2, HD], F16, tag="nrmq")
            rq = rot[:, :, 0:2, :, :].rearrange("p t g h d -> p t g (h d)")
            nc.vector.tensor_mul(
                nrmq, rq, rstdq[:, :, :, None].broadcast_to([128, 4, 2, HD]))
            nc.vector.tensor_mul(
                nrmq, nrmq,
                w2_sb.rearrange("p (g d) -> p g d", g=2)[:, None, :, :]
                .broadcast_to([128, 4, 2, HD]))
            pending[0] = (rot, nrmq, g)

        def attn(b, h, qkT, v_t, rstdk, aT):
            for j in range(T // 512):
                nk = 4 * j + 4
                slab = slab_pool.tile([128, TBB, 512], F16, tag="slab",
                                      name=f"slab{b}_{h}_{j}")
                outT = ps_acc.tile([128, 512], F32, tag="acc",
                                   name=f"outT{b}_{h}_{j}")
                den = ps_acc.tile([128, 512], F32, tag="acc",
                                  name=f"den{b}_{h}_{j}")
                qrhs = qkT[:, h, bass.ds(j * 512, 512)]
                for k in range(nk):
                    st_ps = ps_st.tile([128, 512], F32, tag="st")
                    nc.tensor.matmul(st_ps, qkT[:, 2 + h, bass.ts(k, 128)],
                                     qrhs, start=True, stop=True)
                    nc.scalar.activation(slab[:, k, :], st_ps, AF.Exp,
                                         bias=negc,
                                         scale=rstdk[:, k, h:h + 1])
                    if k >= 4 * j:
                        nc.vector.tensor_mul(slab[:, k, :], slab[:, k, :],
                                             mask_sb[:, k - 4 * j, :])
                    if k >= 1:
                        nc.tensor.matmul(outT, v_t[:, k - 1, bass.ds(h * HD, HD)],
                                         slab[:, k - 1, :],
                                         start=(k == 1), stop=False)
                        nc.tensor.matmul(den, ones_bc, slab[:, k - 1, :],
                                         start=(k == 1), stop=False)
                    if proj_jobs:
                        emit_proj_job(proj_jobs.pop(0))
                nc.tensor.matmul(outT, v_t[:, nk - 1, bass.ds(h * HD, HD)],
                                 slab[:, nk - 1, :], start=False, stop=True)
                nc.tensor.matmul(den, ones_bc, slab[:, nk - 1, :],
                                 start=False, stop=True)
                rec = den_pool.tile([128, 512], F32, tag="rec")
                nc.vector.reciprocal_approx_fast(rec, den)
                nc.vector.tensor_mul(aT[:, h, bass.ds(j * 512, 512)], outT, rec)

        for b in range(B):
            qkT = res.tile([128, 4, T], F16, name=f"qkT{b}", tag="qkT")
            v_t = res.tile([128, TBB, NC], F16, name=f"v{b}", tag="v")
            rstdk = res.tile([128, TBB, NHC], F32, name=f"rstdk{b}", tag="rstdk")
            aT = res.tile([128, NHC, T], F16, name=f"aT{b}", tag=f"aT{b % 2}")
            pending = [None]
            for g in range(TBB // 4):
                qkv_group(b, g, qkT, v_t, rstdk, pending)
            if pending[0] is not None:
                emit_group_transposes(pending[0], qkT)
                pending[0] = None
            for h in range(NHC):
                attn(b, h, qkT, v_t, rstdk, aT)
            for cb in range(HID // 128):
                for tg in range(4):
                    proj_jobs.append((b, cb, tg, aT))
        while proj_jobs:
            emit_proj_job(proj_jobs.pop(0))

    nc.compile()
    return nc


_CACHE = {}


def _get_program():
    if "nc" not in _CACHE:
        _CACHE["nc"] = build_program()
    return _CACHE["nc"]


def _host_tables():
    inv = 1.0 / (ROPE_BASE ** (np.arange(0, HD, 2, dtype=np.float32) / HD))
    freqs = np.arange(T, dtype=np.float32)[:, None] * inv[None, :]
    cos = np.cos(freqs).astype(np.float16)
    sin = np.sin(freqs).astype(np.float16)
    m = np.zeros((4, 128, 512), dtype=np.float16)
    s_idx = np.arange(128)[:, None]
    t_idx = np.arange(512)[None, :]
    for off in range(4):
        m[off] = ((off * 128 + s_idx) <= t_idx).astype(np.float16)
    return cos, sin, m


def kernel(x, Wq, Wk, Wv, Wo, q_rms_w, k_rms_w, **_):
    nc = _get_program()
    cos, sin, masks = _host_tables()
    xT = np.ascontiguousarray(
        np.asarray(x, dtype=np.float32).reshape(TM, HID).T).astype(np.float16)
    w2 = (np.asarray(q_rms_w, dtype=np.float32)
          * np.asarray(k_rms_w, dtype=np.float32) * np.sqrt(HD))
    w2_b = np.ascontiguousarray(
        np.broadcast_to(np.tile(w2, NHC)[None, :], (128, NC))).astype(np.float16)
    ident_h = np.eye(128, dtype=np.float16)

    in_maps = []
    for c in range(N_CORES):
        cols = slice(c * NC, (c + 1) * NC)
        in_maps.append({
            "xT": xT,
            "wqkv": np.ascontiguousarray(
                np.concatenate([Wq[:, cols], Wk[:, cols], Wv[:, cols]], axis=1)
            ).astype(np.float16),
            "wo": np.ascontiguousarray(Wo[cols, :]).astype(np.float16),
            "cos": cos, "sin": sin, "w2": w2_b, "masks": masks,
            "ident": ident_h,
        })

    res = run_bass_kernel_spmd(nc, in_maps, list(range(N_CORES)))
    out = res.results[0]["y"].astype(np.float32)
    for c in range(1, N_CORES):
        out += res.results[c]["y"]
    return np.ascontiguousarray(out.T).reshape(B, T, HID).astype(np.float32)


# revision 14
# speedup vs baseline: 1.2687x; 1.0465x over previous
"""Causal self-attention (B=4, T=2048, HID=2048, NH=16, HD=128) on 8 TRN2 cores.

Tensor-parallel over heads (2 heads/core). v2 redesign vs baseline:
  - No denominator matmuls on PE: P blocks land in a per-j slab
    [128, nk, 512]; den = DVE strided reduce over k-blocks + GPSIMD
    partition_all_reduce (broadcast f32), reciprocal_approx_fast, one mul.
  - K's RMSNorm folded into the Exp activation's per-partition scale
    (rstd_k AP), so kT is stored un-normalized; q gets rstd_q*wqk*sqrt(HD).
  - rstd = exp(-0.5*ln(ssq)) on ScalarE: Ln/Exp/Copy all live in one
    activation table -> no ACT_TABLE_LOAD thrash (was 145us).
  - Per-group (4 t-block) batched DVE rope/square/reduce in fp16.
  - Fused transpose evacuation: 4 PE transposes into one [128,4,128] psum
    tile, single strided copy into the combined qkT resident.
  - proj matmuls of batch b-1 interleaved into attn(b)'s exp-paced PE
    bubbles; aT is double-buffered by batch parity to keep this safe.
  - y emitted fp16 (host sums partials in f32).
"""

import sys

if "/opt/trn_rl_repo" not in sys.path:
    sys.path.insert(0, "/opt/trn_rl_repo")

from contextlib import ExitStack

import numpy as np

import concourse.bass as bass
import concourse.tile as tile
from concourse import bacc, mybir
from concourse.bass_utils import run_bass_kernel_spmd

F32 = mybir.dt.float32
F16 = mybir.dt.float16
AF = mybir.ActivationFunctionType
ALU = mybir.AluOpType
AX = mybir.AxisListType
RED = bass.bass_isa.ReduceOp

B, T, HID = 4, 2048, 2048
NH, HD = 16, 128
N_CORES = 8
NHC = NH // N_CORES          # heads per core = 2
NC = NHC * HD                # per-core head cols = 256
TM = B * T
TBB = T // 128               # 16 t-blocks per batch
KC = HID // 128              # 16 contraction chunks
ROPE_BASE = 10000.0
EXP_BIAS = -1.25


def build_program():
    nc = bacc.Bacc("TRN2", target_bir_lowering=False, debug=False,
                   num_devices=N_CORES)

    xT = nc.dram_tensor("xT", [HID, TM], F16, kind="ExternalInput").ap()
    wqkvd = nc.dram_tensor("wqkv", [HID, 2 * NC + NC], F16,
                           kind="ExternalInput").ap()
    wod = nc.dram_tensor("wo", [NC, HID], F16, kind="ExternalInput").ap()
    cosd = nc.dram_tensor("cos", [T, HD // 2], F16, kind="ExternalInput").ap()
    sind = nc.dram_tensor("sin", [T, HD // 2], F16, kind="ExternalInput").ap()
    w2d = nc.dram_tensor("w2", [128, NC], F16, kind="ExternalInput").ap()
    maskd = nc.dram_tensor("masks", [4, 128, 512], F16, kind="ExternalInput").ap()
    identd = nc.dram_tensor("ident", [128, 128], F16, kind="ExternalInput").ap()
    y = nc.dram_tensor("y", [HID, TM], F16, kind="ExternalOutput").ap()

    with tile.TileContext(nc) as tc, ExitStack() as ctx:
        consts = ctx.enter_context(tc.tile_pool(name="consts", bufs=1))

        wqkv_sb = consts.tile([128, KC, 3 * NC], F16, tag="wqkv")
        nc.sync.dma_start(
            out=wqkv_sb, in_=wqkvd.rearrange("(k1 k2) n -> k2 k1 n", k2=128))
        ident = consts.tile([128, 128], F16, tag="ident")
        nc.sync.dma_start(out=ident, in_=identd)
        cos_sb = consts.tile([128, TBB, HD // 2], F16, tag="cos")
        sin_sb = consts.tile([128, TBB, HD // 2], F16, tag="sin")
        nc.sync.dma_start(out=cos_sb,
                          in_=cosd.rearrange("(t1 t2) j -> t2 t1 j", t2=128))
        nc.sync.dma_start(out=sin_sb,
                          in_=sind.rearrange("(t1 t2) j -> t2 t1 j", t2=128))
        w2_sb = consts.tile([128, NC], F16, tag="w2")
        nc.sync.dma_start(out=w2_sb, in_=w2d)
        mask_sb = consts.tile([128, 4, 512], F16, tag="mask")
        nc.sync.dma_start(out=mask_sb, in_=maskd.rearrange("m p t -> p m t"))
        # wo is not needed until the first proj job (~1/4 into the kernel);
        # load it after everything else so the first x tile isn't delayed.
        wo_sb = consts.tile([128, NHC, HID], F16, tag="wo")
        nc.sync.dma_start(
            out=wo_sb, in_=wod.rearrange("(n1 n2) c -> n2 n1 c", n2=128))
        zero_b = consts.tile([128, 1], F32, tag="zb")
        nc.vector.memset(zero_b, 0.0)
        negc = consts.tile([128, 1], F32, tag="negc")
        nc.vector.memset(negc, EXP_BIAS)
        ones_bc = consts.tile([128, 128], F16, tag="ones")
        nc.vector.memset(ones_bc, 1.0)

        ps_qkv = ctx.enter_context(tc.tile_pool(name="ps_qkv", bufs=2, space="PSUM"))
        ps_tr = ctx.enter_context(tc.tile_pool(name="ps_tr", bufs=1, space="PSUM"))
        ps_st = ctx.enter_context(tc.tile_pool(name="ps_st", bufs=2, space="PSUM"))
        ps_acc = ctx.enter_context(tc.tile_pool(name="ps_acc", bufs=2, space="PSUM"))
        ps_y = ctx.enter_context(tc.tile_pool(name="ps_y", bufs=1, space="PSUM"))

        res = ctx.enter_context(tc.tile_pool(name="res", bufs=1))
        xt_pool = ctx.enter_context(tc.tile_pool(name="xt", bufs=2))
        g_pool = ctx.enter_context(tc.tile_pool(name="gp", bufs=2))
        slab_pool = ctx.enter_context(tc.tile_pool(name="slab", bufs=2))
        den_pool = ctx.enter_context(tc.tile_pool(name="den", bufs=2))
        y_pool = ctx.enter_context(tc.tile_pool(name="yo", bufs=3))

        proj_jobs = []

        def emit_proj_job(job):
            bb, cb, tg, aT = job
            y_ps = ps_y.tile([128, 512], F32, tag="yacc",
                             name=f"yps{bb}_{cb}_{tg}")
            for n in range(NHC):
                nc.tensor.matmul(y_ps, wo_sb[:, n, bass.ts(cb, 128)],
                                 aT[:, n, bass.ds(tg * 512, 512)],
                                 start=(n == 0), stop=(n == NHC - 1))
            ysb = y_pool.tile([128, 512], F16, tag="ysb",
                              name=f"ysb{bb}_{cb}_{tg}")
            nc.any.tensor_copy(ysb, y_ps)
            nc.sync.dma_start(
                out=y[bass.ts(cb, 128), bass.ds(bb * T + tg * 512, 512)],
                in_=ysb)

        def emit_group_transposes(pend, qkT):
            rot, nrmq, g = pend
            for sub in range(4):
                tbl = 4 * g + sub
                t_ps = ps_tr.tile([128, 4, 128], F16, tag="tr",
                                  name=f"tps{tbl}")
                nc.tensor.transpose(t_ps[:, 0, :], nrmq[:, sub, 0, :], ident)
                nc.tensor.transpose(t_ps[:, 1, :], nrmq[:, sub, 1, :], ident)
                nc.tensor.transpose(
                    t_ps[:, 2, :],
                    rot[:, sub, 2, :, :].rearrange("p h d -> p (h d)"), ident)
                nc.tensor.transpose(
                    t_ps[:, 3, :],
                    rot[:, sub, 3, :, :].rearrange("p h d -> p (h d)"), ident)
                nc.any.tensor_copy(qkT[:, :, bass.ds(tbl * 128, 128)], t_ps)

        def qkv_group(b, g, qkT, v_t, rstdk, pending):
            xt = xt_pool.tile([128, KC, 512], F16, tag="xt")
            nc.sync.dma_start(
                out=xt,
                in_=xT[:, bass.ds((b * TBB + 4 * g) * 128, 512)]
                .rearrange("(k1 k2) t -> k2 k1 t", k2=128))
            qk16 = g_pool.tile([128, 4, 512], F16, tag="qk16")
            for sub in range(4):
                tbl = 4 * g + sub
                qk_ps = ps_qkv.tile([128, 512], F32, tag="ps")
                v_ps = ps_qkv.tile([128, NC], F32, tag="ps")
                for k1 in range(KC):
                    lhs = xt[:, k1, bass.ts(sub, 128)]
                    st, sp = (k1 == 0), (k1 == KC - 1)
                    nc.tensor.matmul(qk_ps, lhs,
                                     wqkv_sb[:, k1, 0:512], start=st, stop=sp)
                    nc.tensor.matmul(v_ps, lhs,
                                     wqkv_sb[:, k1, 512:768], start=st, stop=sp)
                nc.scalar.copy(qk16[:, sub, :], qk_ps)
                nc.scalar.copy(v_t[:, tbl, :], v_ps)
            if pending[0] is not None:
                emit_group_transposes(pending[0], qkT)
                pending[0] = None

            # rope on all 4 t-blocks at once, fp16
            rot = g_pool.tile([128, 4, 4, 2, HD // 2], F16, tag="rot")
            tmp = g_pool.tile([128, 4, 4, HD // 2], F16, tag="tmp")
            v5 = qk16.rearrange("p t (g h d) -> p t g h d", g=4, h=2)
            x1, x2 = v5[:, :, :, 0, :], v5[:, :, :, 1, :]
            ct = cos_sb[:, 4 * g:4 * g + 4, None, :].broadcast_to(
                [128, 4, 4, HD // 2])
            sn = sin_sb[:, 4 * g:4 * g + 4, None, :].broadcast_to(
                [128, 4, 4, HD // 2])
            r1 = rot[:, :, :, 0, :]
            r2 = rot[:, :, :, 1, :]
            nc.vector.tensor_mul(r1, x1, ct)
            nc.vector.tensor_mul(tmp, x2, sn)
            nc.vector.tensor_sub(r1, r1, tmp)
            nc.vector.tensor_mul(r2, x2, ct)
            nc.vector.tensor_mul(tmp, x1, sn)
            nc.vector.tensor_add(r2, r2, tmp)

            # ssq per (t-block, tensor-group); rstd = exp(-0.5*ln(ssq))
            sq = g_pool.tile([128, 4, 4, HD], F16, tag="sq")
            rfull = rot.rearrange("p t g h d -> p t g (h d)")
            nc.vector.tensor_mul(sq, rfull, rfull)
            ssq = g_pool.tile([128, 16], F32, tag="ssq")
            nc.vector.tensor_reduce(
                ssq, sq.rearrange("p t g d -> p (t g) d"),
                axis=AX.X, op=ALU.add)
            lssq = g_pool.tile([128, 16], F32, tag="lssq")
            nc.scalar.activation(lssq, ssq, AF.Ln, bias=zero_b)
            lv = lssq.rearrange("p (t g) -> p t g", g=4)
            rstdq = g_pool.tile([128, 4, 2], F32, tag="rstdq")
            nc.scalar.activation(rstdq, lv[:, :, 0:2], AF.Exp, scale=-0.5,
                                 bias=zero_b)
            nc.scalar.activation(rstdk[:, 4 * g:4 * g + 4, :], lv[:, :, 2:4],
                                 AF.Exp, scale=-0.5, bias=zero_b)

            # q_hat = rope(q) * rstd_q * (wq*wk*sqrt(HD))
            nrmq = g_pool.tile([128, 4, 2, HD], F16, tag="nrmq")
            rq = rot[:, :, 0:2, :, :].rearrange("p t g h d -> p t g (h d)")
            nc.vector.tensor_mul(
                nrmq, rq, rstdq[:, :, :, None].broadcast_to([128, 4, 2, HD]))
            nc.vector.tensor_mul(
                nrmq, nrmq,
                w2_sb.rearrange("p (g d) -> p g d", g=2)[:, None, :, :]
                .broadcast_to([128, 4, 2, HD]))
            pending[0] = (rot, nrmq, g)

        def attn(b, h, qkT, v_t, rstdk, aT):
            for j in range(T // 512):
                nk = 4 * j + 4
                slab = slab_pool.tile([128, TBB, 512], F16, tag="slab",
                                      name=f"slab{b}_{h}_{j}")
                outT = ps_acc.tile([128, 512], F32, tag="acc",
                                   name=f"outT{b}_{h}_{j}")
                den = ps_acc.tile([128, 512], F32, tag="acc",
                                  name=f"den{b}_{h}_{j}")
                qrhs = qkT[:, h, bass.ds(j * 512, 512)]
                for k in range(nk):
                    st_ps = ps_st.tile([128, 512], F32, tag="st")
                    nc.tensor.matmul(st_ps, qkT[:, 2 + h, bass.ts(k, 128)],
                                     qrhs, start=True, stop=True)
                    nc.scalar.activation(slab[:, k, :], st_ps, AF.Exp,
                                         bias=negc,
                                         scale=rstdk[:, k, h:h + 1])
                    if k >= 4 * j:
                        nc.vector.tensor_mul(slab[:, k, :], slab[:, k, :],
                                             mask_sb[:, k - 4 * j, :])
                    if k >= 1:
                        nc.tensor.matmul(outT, v_t[:, k - 1, bass.ds(h * HD, HD)],
                                         slab[:, k - 1, :],
                                         start=(k == 1), stop=False)
                        nc.tensor.matmul(den, ones_bc, slab[:, k - 1, :],
                                         start=(k == 1), stop=False)
                    if proj_jobs:
                        emit_proj_job(proj_jobs.pop(0))
                nc.tensor.matmul(outT, v_t[:, nk - 1, bass.ds(h * HD, HD)],
                                 slab[:, nk - 1, :], start=False, stop=True)
                nc.tensor.matmul(den, ones_bc, slab[:, nk - 1, :],
                                 start=False, stop=True)
                rec = den_pool.tile([128, 512], F32, tag="rec")
                nc.vector.reciprocal_approx_fast(rec, den)
                nc.vector.tensor_mul(aT[:, h, bass.ds(j * 512, 512)], outT, rec)
                if h == NHC - 1:
                    # both heads' aT columns for tg=j are now complete; queue
                    # this batch's proj work so it fills attn's PE bubbles
                    for cb in range(HID // 128):
                        proj_jobs.append((b, cb, j, aT))

        for b in range(B):
            qkT = res.tile([128, 4, T], F16, name=f"qkT{b}", tag="qkT")
            v_t = res.tile([128, TBB, NC], F16, name=f"v{b}", tag="v")
            rstdk = res.tile([128, TBB, NHC], F32, name=f"rstdk{b}", tag="rstdk")
            aT = res.tile([128, NHC, T], F16, name=f"aT{b}", tag=f"aT{b % 2}")
            pending = [None]
            for g in range(TBB // 4):
                qkv_group(b, g, qkT, v_t, rstdk, pending)
            if pending[0] is not None:
                emit_group_transposes(pending[0], qkT)
                pending[0] = None
            for h in range(NHC):
                attn(b, h, qkT, v_t, rstdk, aT)
        while proj_jobs:
            emit_proj_job(proj_jobs.pop(0))

    nc.compile()
    return nc


_CACHE = {}


def _get_program():
    if "nc" not in _CACHE:
        _CACHE["nc"] = build_program()
    return _CACHE["nc"]


def _host_tables():
    inv = 1.0 / (ROPE_BASE ** (np.arange(0, HD, 2, dtype=np.float32) / HD))
    freqs = np.arange(T, dtype=np.float32)[:, None] * inv[None, :]
    cos = np.cos(freqs).astype(np.float16)
    sin = np.sin(freqs).astype(np.float16)
    m = np.zeros((4, 128, 512), dtype=np.float16)
    s_idx = np.arange(128)[:, None]
    t_idx = np.arange(512)[None, :]
    for off in range(4):
        m[off] = ((off * 128 + s_idx) <= t_idx).astype(np.float16)
    return cos, sin, m


def kernel(x, Wq, Wk, Wv, Wo, q_rms_w, k_rms_w, **_):
    nc = _get_program()
    cos, sin, masks = _host_tables()
    xT = np.ascontiguousarray(
        np.asarray(x, dtype=np.float32).reshape(TM, HID).T).astype(np.float16)
    w2 = (np.asarray(q_rms_w, dtype=np.float32)
          * np.asarray(k_rms_w, dtype=np.float32) * np.sqrt(HD))
    w2_b = np.ascontiguousarray(
        np.broadcast_to(np.tile(w2, NHC)[None, :], (128, NC))).astype(np.float16)
    ident_h = np.eye(128, dtype=np.float16)

    in_maps = []
    for c in range(N_CORES):
        cols = slice(c * NC, (c + 1) * NC)
        in_maps.append({
            "xT": xT,
            "wqkv": np.ascontiguousarray(
                np.concatenate([Wq[:, cols], Wk[:, cols], Wv[:, cols]], axis=1)
            ).astype(np.float16),
            "wo": np.ascontiguousarray(Wo[cols, :]).astype(np.float16),
            "cos": cos, "sin": sin, "w2": w2_b, "masks": masks,
            "ident": ident_h,
        })

    res = run_bass_kernel_spmd(nc, in_maps, list(range(N_CORES)))
    out = res.results[0]["y"].astype(np.float32)
    for c in range(1, N_CORES):
        out += res.results[c]["y"]
    return np.ascontiguousarray(out.T).reshape(B, T, HID).astype(np.float32)


# revision 18
# speedup vs baseline: 1.3300x; 1.0483x over previous
"""Causal self-attention (B=4, T=2048, HID=2048, NH=16, HD=128) on 8 TRN2 cores.

Tensor-parallel over heads (2 heads/core). v2 redesign vs baseline:
  - No denominator matmuls on PE: P blocks land in a per-j slab
    [128, nk, 512]; den = DVE strided reduce over k-blocks + GPSIMD
    partition_all_reduce (broadcast f32), reciprocal_approx_fast, one mul.
  - K's RMSNorm folded into the Exp activation's per-partition scale
    (rstd_k AP), so kT is stored un-normalized; q gets rstd_q*wqk*sqrt(HD).
  - rstd = exp(-0.5*ln(ssq)) on ScalarE: Ln/Exp/Copy all live in one
    activation table -> no ACT_TABLE_LOAD thrash (was 145us).
  - Per-group (4 t-block) batched DVE rope/square/reduce in fp16.
  - Fused transpose evacuation: 4 PE transposes into one [128,4,128] psum
    tile, single strided copy into the combined qkT resident.
  - proj matmuls of batch b-1 interleaved into attn(b)'s exp-paced PE
    bubbles; aT is double-buffered by batch parity to keep this safe.
  - y emitted fp16 (host sums partials in f32).
"""

import sys

if "/opt/trn_rl_repo" not in sys.path:
    sys.path.insert(0, "/opt/trn_rl_repo")

from contextlib import ExitStack

import numpy as np

import concourse.bass as bass
import concourse.tile as tile
from concourse import bacc, mybir
from concourse.bass_utils import run_bass_kernel_spmd

F32 = mybir.dt.float32
F16 = mybir.dt.float16
AF = mybir.ActivationFunctionType
ALU = mybir.AluOpType
AX = mybir.AxisListType
RED = bass.bass_isa.ReduceOp

B, T, HID = 4, 2048, 2048
NH, HD = 16, 128
N_CORES = 8
NHC = NH // N_CORES          # heads per core = 2
NC = NHC * HD                # per-core head cols = 256
TM = B * T
TBB = T // 128               # 16 t-blocks per batch
KC = HID // 128              # 16 contraction chunks
ROPE_BASE = 10000.0
EXP_BIAS = -1.25


def build_program():
    nc = bacc.Bacc("TRN2", target_bir_lowering=False, debug=False,
                   num_devices=N_CORES)

    xT = nc.dram_tensor("xT", [HID, TM], F16, kind="ExternalInput").ap()
    wqkvd = nc.dram_tensor("wqkv", [HID, 2 * NC + NC], F16,
                           kind="ExternalInput").ap()
    wod = nc.dram_tensor("wo", [NC, HID], F16, kind="ExternalInput").ap()
    cosd = nc.dram_tensor("cos", [T, HD // 2], F16, kind="ExternalInput").ap()
    sind = nc.dram_tensor("sin", [T, HD // 2], F16, kind="ExternalInput").ap()
    w2d = nc.dram_tensor("w2", [128, NC], F16, kind="ExternalInput").ap()
    maskd = nc.dram_tensor("masks", [4, 128, 512], F16, kind="ExternalInput").ap()
    identd = nc.dram_tensor("ident", [128, 128], F16, kind="ExternalInput").ap()
    y = nc.dram_tensor("y", [HID, TM], F16, kind="ExternalOutput").ap()

    with tile.TileContext(nc) as tc, ExitStack() as ctx:
        consts = ctx.enter_context(tc.tile_pool(name="consts", bufs=1))
        xt_pool = ctx.enter_context(tc.tile_pool(name="xt", bufs=2))

        # first x tile before the bulky consts so the first matmul isn't
        # gated on the whole ~7MB constant upload
        xt0 = xt_pool.tile([128, KC, 512], F16, tag="xt", name="xt00")
        nc.sync.dma_start(
            out=xt0,
            in_=xT[:, 0:512].rearrange("(k1 k2) t -> k2 k1 t", k2=128))
        wqkv_sb = consts.tile([128, KC, 3 * NC], F16, tag="wqkv")
        nc.sync.dma_start(
            out=wqkv_sb, in_=wqkvd.rearrange("(k1 k2) n -> k2 k1 n", k2=128))
        ident = consts.tile([128, 128], F16, tag="ident")
        nc.sync.dma_start(out=ident, in_=identd)
        cos_sb = consts.tile([128, TBB, HD // 2], F16, tag="cos")
        sin_sb = consts.tile([128, TBB, HD // 2], F16, tag="sin")
        nc.sync.dma_start(out=cos_sb,
                          in_=cosd.rearrange("(t1 t2) j -> t2 t1 j", t2=128))
        nc.sync.dma_start(out=sin_sb,
                          in_=sind.rearrange("(t1 t2) j -> t2 t1 j", t2=128))
        w2_sb = consts.tile([128, NC], F16, tag="w2")
        nc.sync.dma_start(out=w2_sb, in_=w2d)
        mask_sb = consts.tile([128, 4, 512], F16, tag="mask")
        nc.sync.dma_start(out=mask_sb, in_=maskd.rearrange("m p t -> p m t"))
        # wo is not needed until the first proj job (~1/4 into the kernel);
        # load it after everything else so the first x tile isn't delayed.
        wo_sb = consts.tile([128, NHC, HID], F16, tag="wo")
        nc.sync.dma_start(
            out=wo_sb, in_=wod.rearrange("(n1 n2) c -> n2 n1 c", n2=128))
        zero_b = consts.tile([128, 1], F32, tag="zb")
        nc.vector.memset(zero_b, 0.0)
        negc = consts.tile([128, 1], F32, tag="negc")
        nc.vector.memset(negc, EXP_BIAS)
        ones_bc = consts.tile([128, 128], F16, tag="ones")
        nc.vector.memset(ones_bc, 1.0)

        ps_qkv = ctx.enter_context(tc.tile_pool(name="ps_qkv", bufs=2, space="PSUM"))
        ps_tr = ctx.enter_context(tc.tile_pool(name="ps_tr", bufs=1, space="PSUM"))
        ps_st = ctx.enter_context(tc.tile_pool(name="ps_st", bufs=2, space="PSUM"))
        ps_acc = ctx.enter_context(tc.tile_pool(name="ps_acc", bufs=2, space="PSUM"))
        ps_y = ctx.enter_context(tc.tile_pool(name="ps_y", bufs=1, space="PSUM"))

        res = ctx.enter_context(tc.tile_pool(name="res", bufs=1))
        g_pool = ctx.enter_context(tc.tile_pool(name="gp", bufs=2))
        slab_pool = ctx.enter_context(tc.tile_pool(name="slab", bufs=2))
        den_pool = ctx.enter_context(tc.tile_pool(name="den", bufs=2))
        y_pool = ctx.enter_context(tc.tile_pool(name="yo", bufs=3))

        proj_jobs = []

        def emit_proj_job(job):
            bb, cb, tg, aT = job
            y_ps = ps_y.tile([128, 512], F32, tag="yacc",
                             name=f"yps{bb}_{cb}_{tg}")
            for n in range(NHC):
                nc.tensor.matmul(y_ps, wo_sb[:, n, bass.ts(cb, 128)],
                                 aT[:, n, bass.ds(tg * 512, 512)],
                                 start=(n == 0), stop=(n == NHC - 1))
            ysb = y_pool.tile([128, 512], F16, tag="ysb",
                              name=f"ysb{bb}_{cb}_{tg}")
            nc.any.tensor_copy(ysb, y_ps)
            nc.sync.dma_start(
                out=y[bass.ts(cb, 128), bass.ds(bb * T + tg * 512, 512)],
                in_=ysb)

        def emit_group_transposes(pend, qkT):
            rot, nrmq, g = pend
            for sub in range(4):
                tbl = 4 * g + sub
                t_ps = ps_tr.tile([128, 4, 128], F16, tag="tr",
                                  name=f"tps{tbl}")
                nc.tensor.transpose(t_ps[:, 0, :], nrmq[:, sub, 0, :], ident)
                nc.tensor.transpose(t_ps[:, 1, :], nrmq[:, sub, 1, :], ident)
                nc.tensor.transpose(
                    t_ps[:, 2, :],
                    rot[:, sub, 2, :, :].rearrange("p h d -> p (h d)"), ident)
                nc.tensor.transpose(
                    t_ps[:, 3, :],
                    rot[:, sub, 3, :, :].rearrange("p h d -> p (h d)"), ident)
                nc.any.tensor_copy(qkT[:, :, bass.ds(tbl * 128, 128)], t_ps)

        def qkv_group(b, g, qkT, v_t, rstdk, pending, xt_pre=None):
            if xt_pre is not None:
                xt = xt_pre
            else:
                xt = xt_pool.tile([128, KC, 512], F16, tag="xt")
                nc.sync.dma_start(
                    out=xt,
                    in_=xT[:, bass.ds((b * TBB + 4 * g) * 128, 512)]
                    .rearrange("(k1 k2) t -> k2 k1 t", k2=128))
            qk16 = g_pool.tile([128, 4, 512], F16, tag="qk16")
            for sub in range(4):
                tbl = 4 * g + sub
                qk_ps = ps_qkv.tile([128, 512], F32, tag="ps")
                v_ps = ps_qkv.tile([128, NC], F32, tag="ps")
                for k1 in range(KC):
                    lhs = xt[:, k1, bass.ts(sub, 128)]
                    st, sp = (k1 == 0), (k1 == KC - 1)
                    nc.tensor.matmul(qk_ps, lhs,
                                     wqkv_sb[:, k1, 0:512], start=st, stop=sp)
                    nc.tensor.matmul(v_ps, lhs,
                                     wqkv_sb[:, k1, 512:768], start=st, stop=sp)
                nc.scalar.copy(qk16[:, sub, :], qk_ps)
                nc.scalar.copy(v_t[:, tbl, :], v_ps)
            if pending[0] is not None:
                emit_group_transposes(pending[0], qkT)
                pending[0] = None

            # rope on all 4 t-blocks at once, fp16
            rot = g_pool.tile([128, 4, 4, 2, HD // 2], F16, tag="rot")
            tmp = g_pool.tile([128, 4, 4, HD // 2], F16, tag="tmp")
            v5 = qk16.rearrange("p t (g h d) -> p t g h d", g=4, h=2)
            x1, x2 = v5[:, :, :, 0, :], v5[:, :, :, 1, :]
            ct = cos_sb[:, 4 * g:4 * g + 4, None, :].broadcast_to(
                [128, 4, 4, HD // 2])
            sn = sin_sb[:, 4 * g:4 * g + 4, None, :].broadcast_to(
                [128, 4, 4, HD // 2])
            r1 = rot[:, :, :, 0, :]
            r2 = rot[:, :, :, 1, :]
            nc.vector.tensor_mul(r1, x1, ct)
            nc.vector.tensor_mul(tmp, x2, sn)
            nc.vector.tensor_sub(r1, r1, tmp)
            nc.vector.tensor_mul(r2, x2, ct)
            nc.vector.tensor_mul(tmp, x1, sn)
            nc.vector.tensor_add(r2, r2, tmp)

            # ssq per (t-block, tensor-group); rstd = exp(-0.5*ln(ssq))
            sq = g_pool.tile([128, 4, 4, HD], F16, tag="sq")
            rfull = rot.rearrange("p t g h d -> p t g (h d)")
            nc.vector.tensor_mul(sq, rfull, rfull)
            ssq = g_pool.tile([128, 16], F32, tag="ssq")
            nc.vector.tensor_reduce(
                ssq, sq.rearrange("p t g d -> p (t g) d"),
                axis=AX.X, op=ALU.add)
            lssq = g_pool.tile([128, 16], F32, tag="lssq")
            nc.scalar.activation(lssq, ssq, AF.Ln, bias=zero_b)
            lv = lssq.rearrange("p (t g) -> p t g", g=4)
            rstdq = g_pool.tile([128, 4, 2], F32, tag="rstdq")
            nc.scalar.activation(rstdq, lv[:, :, 0:2], AF.Exp, scale=-0.5,
                                 bias=zero_b)
            nc.scalar.activation(rstdk[:, 4 * g:4 * g + 4, :], lv[:, :, 2:4],
                                 AF.Exp, scale=-0.5, bias=zero_b)

            # q_hat = rope(q) * rstd_q * (wq*wk*sqrt(HD))
            nrmq = g_pool.tile([128, 4, 2, HD], F16, tag="nrmq")
            rq = rot[:, :, 0:2, :, :].rearrange("p t g h d -> p t g (h d)")
            nc.vector.tensor_mul(
                nrmq, rq, rstdq[:, :, :, None].broadcast_to([128, 4, 2, HD]))
            nc.vector.tensor_mul(
                nrmq, nrmq,
                w2_sb.rearrange("p (g d) -> p g d", g=2)[:, None, :, :]
                .broadcast_to([128, 4, 2, HD]))
            pending[0] = (rot, nrmq, g)

        def attn(b, j, h, qkT, v_t, rstdk, aT):
                nk = 4 * j + 4
                slab = slab_pool.tile([128, TBB, 512], F16, tag="slab",
                                      name=f"slab{b}_{h}_{j}")
                outT = ps_acc.tile([128, 512], F32, tag="acc",
                                   name=f"outT{b}_{h}_{j}")
                den = ps_acc.tile([128, 512], F32, tag="acc",
                                  name=f"den{b}_{h}_{j}")
                qrhs = qkT[:, h, bass.ds(j * 512, 512)]
                for k in range(nk):
                    st_ps = ps_st.tile([128, 512], F32, tag="st")
                    nc.tensor.matmul(st_ps, qkT[:, 2 + h, bass.ts(k, 128)],
                                     qrhs, start=True, stop=True)
                    nc.scalar.activation(slab[:, k, :], st_ps, AF.Exp,
                                         bias=negc,
                                         scale=rstdk[:, k, h:h + 1])
                    if k >= 4 * j:
                        nc.vector.tensor_mul(slab[:, k, :], slab[:, k, :],
                                             mask_sb[:, k - 4 * j, :])
                    if k >= 1:
                        nc.tensor.matmul(outT, v_t[:, k - 1, bass.ds(h * HD, HD)],
                                         slab[:, k - 1, :],
                                         start=(k == 1), stop=False)
                        nc.tensor.matmul(den, ones_bc, slab[:, k - 1, :],
                                         start=(k == 1), stop=False)
                    if proj_jobs:
                        emit_proj_job(proj_jobs.pop(0))
                nc.tensor.matmul(outT, v_t[:, nk - 1, bass.ds(h * HD, HD)],
                                 slab[:, nk - 1, :], start=False, stop=True)
                nc.tensor.matmul(den, ones_bc, slab[:, nk - 1, :],
                                 start=False, stop=True)
                rec = den_pool.tile([128, 512], F32, tag="rec")
                nc.vector.reciprocal_approx_fast(rec, den)
                nc.vector.tensor_mul(aT[:, h, bass.ds(j * 512, 512)], outT, rec)
                if h == NHC - 1:
                    # both heads' aT columns for tg=j are now complete; queue
                    # this batch's proj work so it fills attn's PE bubbles
                    for cb in range(HID // 128):
                        proj_jobs.append((b, cb, j, aT))

        for b in range(B):
            qkT = res.tile([128, 4, T], F16, name=f"qkT{b}", tag="qkT")
            v_t = res.tile([128, TBB, NC], F16, name=f"v{b}", tag="v")
            rstdk = res.tile([128, TBB, NHC], F32, name=f"rstdk{b}", tag="rstdk")
            aT = res.tile([128, NHC, T], F16, name=f"aT{b}", tag=f"aT{b % 2}")
            pending = [None]
            for g in range(TBB // 4):
                qkv_group(b, g, qkT, v_t, rstdk, pending,
                          xt_pre=xt0 if (b == 0 and g == 0) else None)
            if pending[0] is not None:
                emit_group_transposes(pending[0], qkT)
                pending[0] = None
            # j-major so both heads' aT columns for tg=j finish early and
            # that column's proj jobs can interleave into the remaining attn
            for j in range(T // 512):
                for h in range(NHC):
                    attn(b, j, h, qkT, v_t, rstdk, aT)
        while proj_jobs:
            emit_proj_job(proj_jobs.pop(0))

    nc.compile()
    return nc


_CACHE = {}


def _get_program():
    if "nc" not in _CACHE:
        _CACHE["nc"] = build_program()
    return _CACHE["nc"]


def _host_tables():
    inv = 1.0 / (ROPE_BASE ** (np.arange(0, HD, 2, dtype=np.float32) / HD))
    freqs = np.arange(T, dtype=np.float32)[:, None] * inv[None, :]
    cos = np.cos(freqs).astype(np.float16)
    sin = np.sin(freqs).astype(np.float16)
    m = np.zeros((4, 128, 512), dtype=np.float16)
    s_idx = np.arange(128)[:, None]
    t_idx = np.arange(512)[None, :]
    for off in range(4):
        m[off] = ((off * 128 + s_idx) <= t_idx).astype(np.float16)
    return cos, sin, m


def kernel(x, Wq, Wk, Wv, Wo, q_rms_w, k_rms_w, **_):
    nc = _get_program()
    cos, sin, masks = _host_tables()
    xT = np.ascontiguousarray(
        np.asarray(x, dtype=np.float32).reshape(TM, HID).T).astype(np.float16)
    w2 = (np.asarray(q_rms_w, dtype=np.float32)
          * np.asarray(k_rms_w, dtype=np.float32) * np.sqrt(HD))
    w2_b = np.ascontiguousarray(
        np.broadcast_to(np.tile(w2, NHC)[None, :], (128, NC))).astype(np.float16)
    ident_h = np.eye(128, dtype=np.float16)

    in_maps = []
    for c in range(N_CORES):
        cols = slice(c * NC, (c + 1) * NC)
        in_maps.append({
            "xT": xT,
            "wqkv": np.ascontiguousarray(
                np.concatenate([Wq[:, cols], Wk[:, cols], Wv[:, cols]], axis=1)
            ).astype(np.float16),
            "wo": np.ascontiguousarray(Wo[cols, :]).astype(np.float16),
            "cos": cos, "sin": sin, "w2": w2_b, "masks": masks,
            "ident": ident_h,
        })

    res = run_bass_kernel_spmd(nc, in_maps, list(range(N_CORES)))
    out = res.results[0]["y"].astype(np.float32)
    for c in range(1, N_CORES):
        out += res.results[c]["y"]
    return np.ascontiguousarray(out.T).reshape(B, T, HID).astype(np.float32)


# revision 19
# speedup vs baseline: 1.3335x; 1.0026x over previous
"""Causal self-attention (B=4, T=2048, HID=2048, NH=16, HD=128) on 8 TRN2 cores.

Tensor-parallel over heads (2 heads/core). v2 redesign vs baseline:
  - No denominator matmuls on PE: P blocks land in a per-j slab
    [128, nk, 512]; den = DVE strided reduce over k-blocks + GPSIMD
    partition_all_reduce (broadcast f32), reciprocal_approx_fast, one mul.
  - K's RMSNorm folded into the Exp activation's per-partition scale
    (rstd_k AP), so kT is stored un-normalized; q gets rstd_q*wqk*sqrt(HD).
  - rstd = exp(-0.5*ln(ssq)) on ScalarE: Ln/Exp/Copy all live in one
    activation table -> no ACT_TABLE_LOAD thrash (was 145us).
  - Per-group (4 t-block) batched DVE rope/square/reduce in fp16.
  - Fused transpose evacuation: 4 PE transposes into one [128,4,128] psum
    tile, single strided copy into the combined qkT resident.
  - proj matmuls of batch b-1 interleaved into attn(b)'s exp-paced PE
    bubbles; aT is double-buffered by batch parity to keep this safe.
  - y emitted fp16 (host sums partials in f32).
"""

import sys

if "/opt/trn_rl_repo" not in sys.path:
    sys.path.insert(0, "/opt/trn_rl_repo")

from contextlib import ExitStack

import numpy as np

import concourse.bass as bass
import concourse.tile as tile
from concourse import bacc, mybir
from concourse.bass_utils import run_bass_kernel_spmd

F32 = mybir.dt.float32
F16 = mybir.dt.float16
AF = mybir.ActivationFunctionType
ALU = mybir.AluOpType
AX = mybir.AxisListType
RED = bass.bass_isa.ReduceOp

B, T, HID = 4, 2048, 2048
NH, HD = 16, 128
N_CORES = 8
NHC = NH // N_CORES          # heads per core = 2
NC = NHC * HD                # per-core head cols = 256
TM = B * T
TBB = T // 128               # 16 t-blocks per batch
KC = HID // 128              # 16 contraction chunks
ROPE_BASE = 10000.0
EXP_BIAS = -1.25


def build_program():
    nc = bacc.Bacc("TRN2", target_bir_lowering=False, debug=False,
                   num_devices=N_CORES)

    xT = nc.dram_tensor("xT", [HID, TM], F16, kind="ExternalInput").ap()
    wqkvd = nc.dram_tensor("wqkv", [HID, 2 * NC + NC], F16,
                           kind="ExternalInput").ap()
    wod = nc.dram_tensor("wo", [NC, HID], F16, kind="ExternalInput").ap()
    cosd = nc.dram_tensor("cos", [T, HD // 2], F16, kind="ExternalInput").ap()
    sind = nc.dram_tensor("sin", [T, HD // 2], F16, kind="ExternalInput").ap()
    w2d = nc.dram_tensor("w2", [128, NC], F16, kind="ExternalInput").ap()
    maskd = nc.dram_tensor("masks", [4, 128, 512], F16, kind="ExternalInput").ap()
    identd = nc.dram_tensor("ident", [128, 128], F16, kind="ExternalInput").ap()
    y = nc.dram_tensor("y", [HID, TM], F16, kind="ExternalOutput").ap()

    with tile.TileContext(nc) as tc, ExitStack() as ctx:
        consts = ctx.enter_context(tc.tile_pool(name="consts", bufs=1))
        xt_pool = ctx.enter_context(tc.tile_pool(name="xt", bufs=2))

        # first x tile before the bulky consts so the first matmul isn't
        # gated on the whole ~7MB constant upload
        xt0 = xt_pool.tile([128, KC, 512], F16, tag="xt", name="xt00")
        nc.sync.dma_start(
            out=xt0,
            in_=xT[:, 0:512].rearrange("(k1 k2) t -> k2 k1 t", k2=128))
        # split the weight upload so the first 8 contraction chunks (and
        # with them the first matmuls) unblock at half the transfer time
        wqkv_a = consts.tile([128, KC // 2, 3 * NC], F16, tag="wqkva")
        wqkv_b = consts.tile([128, KC // 2, 3 * NC], F16, tag="wqkvb")
        wqkv_r = wqkvd.rearrange("(k1 k2) n -> k2 k1 n", k2=128)
        nc.sync.dma_start(out=wqkv_a, in_=wqkv_r[:, 0:KC // 2, :])
        nc.sync.dma_start(out=wqkv_b, in_=wqkv_r[:, KC // 2:KC, :])
        ident = consts.tile([128, 128], F16, tag="ident")
        nc.sync.dma_start(out=ident, in_=identd)
        cos_sb = consts.tile([128, TBB, HD // 2], F16, tag="cos")
        sin_sb = consts.tile([128, TBB, HD // 2], F16, tag="sin")
        nc.sync.dma_start(out=cos_sb,
                          in_=cosd.rearrange("(t1 t2) j -> t2 t1 j", t2=128))
        nc.sync.dma_start(out=sin_sb,
                          in_=sind.rearrange("(t1 t2) j -> t2 t1 j", t2=128))
        w2_sb = consts.tile([128, NC], F16, tag="w2")
        nc.sync.dma_start(out=w2_sb, in_=w2d)
        mask_sb = consts.tile([128, 4, 512], F16, tag="mask")
        nc.sync.dma_start(out=mask_sb, in_=maskd.rearrange("m p t -> p m t"))
        # wo is not needed until the first proj job (~1/4 into the kernel);
        # load it after everything else so the first x tile isn't delayed.
        wo_sb = consts.tile([128, NHC, HID], F16, tag="wo")
        nc.sync.dma_start(
            out=wo_sb, in_=wod.rearrange("(n1 n2) c -> n2 n1 c", n2=128))
        zero_b = consts.tile([128, 1], F32, tag="zb")
        nc.vector.memset(zero_b, 0.0)
        negc = consts.tile([128, 1], F32, tag="negc")
        nc.vector.memset(negc, EXP_BIAS)
        ones_bc = consts.tile([128, 128], F16, tag="ones")
        nc.vector.memset(ones_bc, 1.0)

        ps_qkv = ctx.enter_context(tc.tile_pool(name="ps_qkv", bufs=2, space="PSUM"))
        ps_tr = ctx.enter_context(tc.tile_pool(name="ps_tr", bufs=1, space="PSUM"))
        ps_st = ctx.enter_context(tc.tile_pool(name="ps_st", bufs=2, space="PSUM"))
        ps_acc = ctx.enter_context(tc.tile_pool(name="ps_acc", bufs=2, space="PSUM"))
        ps_y = ctx.enter_context(tc.tile_pool(name="ps_y", bufs=1, space="PSUM"))

        res = ctx.enter_context(tc.tile_pool(name="res", bufs=1))
        g_pool = ctx.enter_context(tc.tile_pool(name="gp", bufs=2))
        slab_pool = ctx.enter_context(tc.tile_pool(name="slab", bufs=2))
        den_pool = ctx.enter_context(tc.tile_pool(name="den", bufs=2))
        y_pool = ctx.enter_context(tc.tile_pool(name="yo", bufs=3))

        proj_jobs = []

        def emit_proj_job(job):
            bb, cb, tg, aT = job
            y_ps = ps_y.tile([128, 512], F32, tag="yacc",
                             name=f"yps{bb}_{cb}_{tg}")
            for n in range(NHC):
                nc.tensor.matmul(y_ps, wo_sb[:, n, bass.ts(cb, 128)],
                                 aT[:, n, bass.ds(tg * 512, 512)],
                                 start=(n == 0), stop=(n == NHC - 1))
            ysb = y_pool.tile([128, 512], F16, tag="ysb",
                              name=f"ysb{bb}_{cb}_{tg}")
            nc.any.tensor_copy(ysb, y_ps)
            nc.sync.dma_start(
                out=y[bass.ts(cb, 128), bass.ds(bb * T + tg * 512, 512)],
                in_=ysb)

        def emit_group_transposes(pend, qkT):
            rot, nrmq, g = pend
            for sub in range(4):
                tbl = 4 * g + sub
                t_ps = ps_tr.tile([128, 4, 128], F16, tag="tr",
                                  name=f"tps{tbl}")
                nc.tensor.transpose(t_ps[:, 0, :], nrmq[:, sub, 0, :], ident)
                nc.tensor.transpose(t_ps[:, 1, :], nrmq[:, sub, 1, :], ident)
                nc.tensor.transpose(
                    t_ps[:, 2, :],
                    rot[:, sub, 2, :, :].rearrange("p h d -> p (h d)"), ident)
                nc.tensor.transpose(
                    t_ps[:, 3, :],
                    rot[:, sub, 3, :, :].rearrange("p h d -> p (h d)"), ident)
                nc.any.tensor_copy(qkT[:, :, bass.ds(tbl * 128, 128)], t_ps)

        def qkv_group(b, g, qkT, v_t, rstdk, pending, xt_pre=None):
            if xt_pre is not None:
                xt = xt_pre
            else:
                xt = xt_pool.tile([128, KC, 512], F16, tag="xt")
                nc.sync.dma_start(
                    out=xt,
                    in_=xT[:, bass.ds((b * TBB + 4 * g) * 128, 512)]
                    .rearrange("(k1 k2) t -> k2 k1 t", k2=128))
            qk16 = g_pool.tile([128, 4, 512], F16, tag="qk16")
            for sub in range(4):
                tbl = 4 * g + sub
                qk_ps = ps_qkv.tile([128, 512], F32, tag="ps")
                v_ps = ps_qkv.tile([128, NC], F32, tag="ps")
                for k1 in range(KC):
                    lhs = xt[:, k1, bass.ts(sub, 128)]
                    st, sp = (k1 == 0), (k1 == KC - 1)
                    wsb = wqkv_a if k1 < KC // 2 else wqkv_b
                    kk = k1 % (KC // 2)
                    nc.tensor.matmul(qk_ps, lhs,
                                     wsb[:, kk, 0:512], start=st, stop=sp)
                    nc.tensor.matmul(v_ps, lhs,
                                     wsb[:, kk, 512:768], start=st, stop=sp)
                nc.scalar.copy(qk16[:, sub, :], qk_ps)
                nc.scalar.copy(v_t[:, tbl, :], v_ps)
            if pending[0] is not None:
                emit_group_transposes(pending[0], qkT)
                pending[0] = None

            # rope on all 4 t-blocks at once, fp16
            rot = g_pool.tile([128, 4, 4, 2, HD // 2], F16, tag="rot")
            tmp = g_pool.tile([128, 4, 4, HD // 2], F16, tag="tmp")
            v5 = qk16.rearrange("p t (g h d) -> p t g h d", g=4, h=2)
            x1, x2 = v5[:, :, :, 0, :], v5[:, :, :, 1, :]
            ct = cos_sb[:, 4 * g:4 * g + 4, None, :].broadcast_to(
                [128, 4, 4, HD // 2])
            sn = sin_sb[:, 4 * g:4 * g + 4, None, :].broadcast_to(
                [128, 4, 4, HD // 2])
            r1 = rot[:, :, :, 0, :]
            r2 = rot[:, :, :, 1, :]
            nc.vector.tensor_mul(r1, x1, ct)
            nc.vector.tensor_mul(tmp, x2, sn)
            nc.vector.tensor_sub(r1, r1, tmp)
            nc.vector.tensor_mul(r2, x2, ct)
            nc.vector.tensor_mul(tmp, x1, sn)
            nc.vector.tensor_add(r2, r2, tmp)

            # ssq per (t-block, tensor-group); rstd = exp(-0.5*ln(ssq))
            sq = g_pool.tile([128, 4, 4, HD], F16, tag="sq")
            rfull = rot.rearrange("p t g h d -> p t g (h d)")
            nc.vector.tensor_mul(sq, rfull, rfull)
            ssq = g_pool.tile([128, 16], F32, tag="ssq")
            nc.vector.tensor_reduce(
                ssq, sq.rearrange("p t g d -> p (t g) d"),
                axis=AX.X, op=ALU.add)
            lssq = g_pool.tile([128, 16], F32, tag="lssq")
            nc.scalar.activation(lssq, ssq, AF.Ln, bias=zero_b)
            lv = lssq.rearrange("p (t g) -> p t g", g=4)
            rstdq = g_pool.tile([128, 4, 2], F32, tag="rstdq")
            nc.scalar.activation(rstdq, lv[:, :, 0:2], AF.Exp, scale=-0.5,
                                 bias=zero_b)
            nc.scalar.activation(rstdk[:, 4 * g:4 * g + 4, :], lv[:, :, 2:4],
                                 AF.Exp, scale=-0.5, bias=zero_b)

            # q_hat = rope(q) * rstd_q * (wq*wk*sqrt(HD))
            nrmq = g_pool.tile([128, 4, 2, HD], F16, tag="nrmq")
            rq = rot[:, :, 0:2, :, :].rearrange("p t g h d -> p t g (h d)")
            nc.vector.tensor_mul(
                nrmq, rq, rstdq[:, :, :, None].broadcast_to([128, 4, 2, HD]))
            nc.vector.tensor_mul(
                nrmq, nrmq,
                w2_sb.rearrange("p (g d) -> p g d", g=2)[:, None, :, :]
                .broadcast_to([128, 4, 2, HD]))
            pending[0] = (rot, nrmq, g)

        def attn(b, j, h, qkT, v_t, rstdk, aT):
                nk = 4 * j + 4
                slab = slab_pool.tile([128, TBB, 512], F16, tag="slab",
                                      name=f"slab{b}_{h}_{j}")
                outT = ps_acc.tile([128, 512], F32, tag="acc",
                                   name=f"outT{b}_{h}_{j}")
                den = ps_acc.tile([128, 512], F32, tag="acc",
                                  name=f"den{b}_{h}_{j}")
                qrhs = qkT[:, h, bass.ds(j * 512, 512)]
                for k in range(nk):
                    st_ps = ps_st.tile([128, 512], F32, tag="st")
                    nc.tensor.matmul(st_ps, qkT[:, 2 + h, bass.ts(k, 128)],
                                     qrhs, start=True, stop=True)
                    nc.scalar.activation(slab[:, k, :], st_ps, AF.Exp,
                                         bias=negc,
                                         scale=rstdk[:, k, h:h + 1])
                    if k >= 4 * j:
                        nc.vector.tensor_mul(slab[:, k, :], slab[:, k, :],
                                             mask_sb[:, k - 4 * j, :])
                    if k >= 1:
                        nc.tensor.matmul(outT, v_t[:, k - 1, bass.ds(h * HD, HD)],
                                         slab[:, k - 1, :],
                                         start=(k == 1), stop=False)
                        nc.tensor.matmul(den, ones_bc, slab[:, k - 1, :],
                                         start=(k == 1), stop=False)
                    if proj_jobs:
                        emit_proj_job(proj_jobs.pop(0))
                nc.tensor.matmul(outT, v_t[:, nk - 1, bass.ds(h * HD, HD)],
                                 slab[:, nk - 1, :], start=False, stop=True)
                nc.tensor.matmul(den, ones_bc, slab[:, nk - 1, :],
                                 start=False, stop=True)
                rec = den_pool.tile([128, 512], F32, tag="rec")
                nc.vector.reciprocal_approx_fast(rec, den)
                nc.vector.tensor_mul(aT[:, h, bass.ds(j * 512, 512)], outT, rec)
                if h == NHC - 1:
                    # both heads' aT columns for tg=j are now complete; queue
                    # this batch's proj work so it fills attn's PE bubbles
                    for cb in range(HID // 128):
                        proj_jobs.append((b, cb, j, aT))

        for b in range(B):
            qkT = res.tile([128, 4, T], F16, name=f"qkT{b}", tag="qkT")
            v_t = res.tile([128, TBB, NC], F16, name=f"v{b}", tag="v")
            rstdk = res.tile([128, TBB, NHC], F32, name=f"rstdk{b}", tag="rstdk")
            aT = res.tile([128, NHC, T], F16, name=f"aT{b}", tag=f"aT{b % 2}")
            pending = [None]
            for g in range(TBB // 4):
                qkv_group(b, g, qkT, v_t, rstdk, pending,
                          xt_pre=xt0 if (b == 0 and g == 0) else None)
            if pending[0] is not None:
                emit_group_transposes(pending[0], qkT)
                pending[0] = None
            # j-major so both heads' aT columns for tg=j finish early and
            # that column's proj jobs can interleave into the remaining attn
            for j in range(T // 512):
                for h in range(NHC):
                    attn(b, j, h, qkT, v_t, rstdk, aT)
        while proj_jobs:
            emit_proj_job(proj_jobs.pop(0))

    nc.compile()
    return nc


_CACHE = {}


def _get_program():
    if "nc" not in _CACHE:
        _CACHE["nc"] = build_program()
    return _CACHE["nc"]


def _host_tables():
    inv = 1.0 / (ROPE_BASE ** (np.arange(0, HD, 2, dtype=np.float32) / HD))
    freqs = np.arange(T, dtype=np.float32)[:, None] * inv[None, :]
    cos = np.cos(freqs).astype(np.float16)
    sin = np.sin(freqs).astype(np.float16)
    m = np.zeros((4, 128, 512), dtype=np.float16)
    s_idx = np.arange(128)[:, None]
    t_idx = np.arange(512)[None, :]
    for off in range(4):
        m[off] = ((off * 128 + s_idx) <= t_idx).astype(np.float16)
    return cos, sin, m


def kernel(x, Wq, Wk, Wv, Wo, q_rms_w, k_rms_w, **_):
    nc = _get_program()
    cos, sin, masks = _host_tables()
    xT = np.ascontiguousarray(
        np.asarray(x, dtype=np.float32).reshape(TM, HID).T).astype(np.float16)
    w2 = (np.asarray(q_rms_w, dtype=np.float32)
          * np.asarray(k_rms_w, dtype=np.float32) * np.sqrt(HD))
    w2_b = np.ascontiguousarray(
        np.broadcast_to(np.tile(w2, NHC)[None, :], (128, NC))).astype(np.float16)
    ident_h = np.eye(128, dtype=np.float16)

    in_maps = []
    for c in range(N_CORES):
        cols = slice(c * NC, (c + 1) * NC)
        in_maps.append({
            "xT": xT,
            "wqkv": np.ascontiguousarray(
                np.concatenate([Wq[:, cols], Wk[:, cols], Wv[:, cols]], axis=1)
            ).astype(np.float16),
            "wo": np.ascontiguousarray(Wo[cols, :]).astype(np.float16),
            "cos": cos, "sin": sin, "w2": w2_b, "masks": masks,
            "ident": ident_h,
        })

    res = run_bass_kernel_spmd(nc, in_maps, list(range(N_CORES)))
    out = res.results[0]["y"].astype(np.float32)
    for c in range(1, N_CORES):
        out += res.results[c]["y"]
    return np.ascontiguousarray(out.T).reshape(B, T, HID).astype(np.float32)
